# revision 16
# baseline (speedup 1.0000x reference)
"""Detection postprocess (decode + top-60 + per-image NMS) on 8 TRN2 NeuronCores.

Data-parallel over the batch: 256 images -> 32 per core. Per core, one raw-Bass
program (no TileContext; every instruction carries at most one sync wait):

  DVE   : per-chunk top-8 values (max) -> mark cells (match_replace) -> exact
          (value, position) records per chunk via prefix-scan + one-hot
          reductions and integer-position-key max rounds -> per-image top-64
          over the 1024-slot pool (max/match_replace) -> marked-pool positions
          via integer keys -> 20-step NMS over [32,64] lanes (one image per
          partition, all 32 in lockstep, on logits).
  GPSIMD: all DMAs (single SWDGE queue) + per-image gathers via indirect_copy
          (one 16-partition group per image, channels on partitions).
  ACT   : sigmoid of the top-64 logits (emitted scores only; ordering uses
          exact logits).

The pool and its NMS candidate list are ordered by ascending global index,
which reproduces jax top_k / argmax tie-breaking exactly.
"""

import numpy as np

import concourse.bass as bass
from concourse import mybir
from concourse.bass_utils import run_bass_kernel_spmd

dt = mybir.dt
Alu = mybir.AluOpType
AF = mybir.ActivationFunctionType
Ax = mybir.AxisListType

B = 32            # images per core
N = 13824         # anchors per image (24^3)
CH = 108          # chunk length
Q = 128           # chunks per image
TOP = 64          # extracted top-k (top-60 kept, rest masked)
NMSK = 20
NEG = -1e9
NEGINF = -1e30
L0 = float(np.float32(np.log(np.float32(0.15) / np.float32(0.85))))  # logit threshold
THP = float(np.float32(0.05) / np.float32(1.05))  # iou>th  <=>  inter > THP*(v1+v2)


def build_nc(dbg=False):
    nc = bass.Bass("TRN2", target_bir_lowering=False, debug=False, num_devices=8)

    cls = nc.declare_dram_parameter("cls", [B, N], dt.float32, isOutput=False)
    off = nc.declare_dram_parameter("off", [B, 3, N], dt.float32, isOutput=False)
    sh = nc.declare_dram_parameter("sh", [B, 3, N], dt.float32, isOutput=False)
    anc = nc.declare_dram_parameter("anc", [8, 3, N], dt.float32, isOutput=False)
    chb = nc.declare_dram_parameter("chb", [128, 1], dt.float32, isOutput=False)
    jc = nc.declare_dram_parameter("jc", [128, B * CH], dt.float32, isOutput=False)    # 107 - (col%108)
    pp2 = nc.declare_dram_parameter("pp2", [B, Q * 8], dt.float32, isOutput=False)     # 6096 - pos
    outp = nc.declare_dram_parameter("out", [B, 60, 8], dt.float32, isOutput=True)
    dbg_outs = {}
    if dbg:
        for nm, shp, dty in [
            ("d_v1", [128, B * 8], dt.float32), ("d_kp", [128, B * 8], dt.float32),
            ("d_vj", [128, B * 8], dt.float32), ("d_gidxf", [128, B * 8], dt.float32),
            ("d_pool0", [B, Q * 8], dt.float32), ("d_gip", [B, Q * 8], dt.float32),
            ("d_vtop", [B, TOP], dt.float32), ("d_posl", [B, TOP], dt.float32),
            ("d_cv", [B, TOP], dt.float32), ("d_g64", [B, TOP], dt.float32),
            ("d_raw", [B, 9 * TOP], dt.float32), ("d_gs", [B, 8 * TOP], dt.float32),
        ]:
            dbg_outs[nm] = nc.declare_dram_parameter(nm, shp, dty, isOutput=True)

    # DRAM scratch for cross-layout bounces
    scr_vj = nc.dram_tensor("scr_vj", [Q, B, 8], dt.float32)
    scr_gi = nc.dram_tensor("scr_gi", [Q, B, 8], dt.float32)
    scr_gip = nc.dram_tensor("scr_gip", [B, Q * 8], dt.float32)
    scr_p0 = nc.dram_tensor("scr_p0", [B, Q * 8], dt.float32)
    scr_posw = nc.dram_tensor("scr_posw", [B, TOP], dt.uint16)
    scr_o1 = nc.dram_tensor("scr_o1", [128, 4 * TOP], dt.float32)
    scr_gw = nc.dram_tensor("scr_gw", [B, TOP], dt.uint16)
    scr_g2 = nc.dram_tensor("scr_g2", [4, 128, TOP], dt.float32)

    # SBUF -- full-width tiles ([128, 3456] f32 = 13.8KB/partition each)
    T1 = nc.alloc_sbuf_tensor("T1", [128, B * CH], dt.float32)    # [q, (b j)]
    T1R = nc.alloc_sbuf_tensor("T1R", [128, B * CH], dt.float32)  # marked copy, then STT out
    WRK = nc.alloc_sbuf_tensor("WRK", [128, B * CH], dt.float32)  # Kp then TM
    JCT = nc.alloc_sbuf_tensor("JCT", [128, B * CH], dt.float32)  # jc const, then CS
    MKU8 = nc.alloc_sbuf_tensor("MKU8", [128, B * CH], dt.uint8)
    DG = nc.alloc_sbuf_tensor("DG", [128, N], dt.float32)         # gather channels

    # narrow tiles
    CHB = nc.alloc_sbuf_tensor("CHB", [128, 1], dt.float32)
    V1 = nc.alloc_sbuf_tensor("V1", [128, B * 8], dt.float32)
    KP = nc.alloc_sbuf_tensor("KP", [128, B * 8], dt.float32)
    VJ = nc.alloc_sbuf_tensor("VJ", [128, B * 8], dt.float32)
    GIDXF = nc.alloc_sbuf_tensor("GIDXF", [128, B * 8], dt.float32)
    Z1 = nc.alloc_sbuf_tensor("Z1", [128, 1], dt.float32)         # zero, broadcast for scans
    DMY = nc.alloc_sbuf_tensor("DMY", [B, TOP], dt.float32)       # max-latency gap scratch
    POOL = nc.alloc_sbuf_tensor("POOL", [B, Q * 8], dt.float32)
    PP2T = nc.alloc_sbuf_tensor("PP2T", [B, Q * 8], dt.float32)
    K2 = nc.alloc_sbuf_tensor("K2", [B, Q * 8], dt.float32)
    MD2 = nc.alloc_sbuf_tensor("MD2", [B, Q * 8], dt.float32)
    GIP = nc.alloc_sbuf_tensor("GIP", [B, Q * 8], dt.float32)
    VTOP = nc.alloc_sbuf_tensor("VTOP", [B, TOP], dt.float32)
    KT = nc.alloc_sbuf_tensor("KT", [B, TOP], dt.float32)
    POSL = nc.alloc_sbuf_tensor("POSL", [B, TOP], dt.float32)
    POSW = nc.alloc_sbuf_tensor("POSW", [B, TOP], dt.uint16)
    GD = nc.alloc_sbuf_tensor("GD", [128, Q * 8], dt.float32)
    PW1 = nc.alloc_sbuf_tensor("PW1", [128, 4], dt.uint16)
    OUT1 = nc.alloc_sbuf_tensor("OUT1", [128, 4 * TOP], dt.float32)
    PW2 = nc.alloc_sbuf_tensor("PW2", [128, 4], dt.uint16)
    G2 = nc.alloc_sbuf_tensor("G2", [128, TOP], dt.float32)
    CV = nc.alloc_sbuf_tensor("CV", [B, TOP], dt.float32)
    GIDX64F = nc.alloc_sbuf_tensor("GIDX64F", [B, TOP], dt.float32)
    GIDXW = nc.alloc_sbuf_tensor("GIDXW", [B, TOP], dt.uint16)
    RAW = nc.alloc_sbuf_tensor("RAW", [B, 9 * TOP], dt.float32)   # off3|sh3|anc3
    GS = nc.alloc_sbuf_tensor("GS", [B, 8 * TOP], dt.float32)     # C3|S3|V2|SIG
    LOT = nc.alloc_sbuf_tensor("LOT", [B, 3 * TOP], dt.float32)
    HIT = nc.alloc_sbuf_tensor("HIT", [B, 3 * TOP], dt.float32)
    HALF = nc.alloc_sbuf_tensor("HALF", [B, 3 * TOP], dt.float32)
    W = nc.alloc_sbuf_tensor("W", [B, TOP], dt.float32)
    NEGT = nc.alloc_sbuf_tensor("NEGT", [B, TOP], dt.float32)
    GT = nc.alloc_sbuf_tensor("GT", [B, TOP], dt.float32)
    EQ = nc.alloc_sbuf_tensor("EQ", [B, TOP], dt.float32)
    CUM = nc.alloc_sbuf_tensor("CUM", [B, TOP], dt.float32)
    NG = nc.alloc_sbuf_tensor("NG", [B, 1], dt.float32)
    NEED = nc.alloc_sbuf_tensor("NEED", [B, 1], dt.float32)
    OKE = nc.alloc_sbuf_tensor("OKE", [B, TOP], dt.float32)
    KEEP = nc.alloc_sbuf_tensor("KEEP", [B, TOP], dt.float32)
    MU8 = nc.alloc_sbuf_tensor("MU8", [B, TOP], dt.uint8)
    M8 = nc.alloc_sbuf_tensor("M8", [B, 8], dt.float32)
    OHR = nc.alloc_sbuf_tensor("OHR", [B, TOP], dt.float32)
    CSOH = nc.alloc_sbuf_tensor("CSOH", [B, TOP], dt.float32)
    OH = nc.alloc_sbuf_tensor("OH", [B, TOP], dt.float32)
    TMP8 = nc.alloc_sbuf_tensor("TMP8", [B, 8 * TOP], dt.float32)
    G8 = nc.alloc_sbuf_tensor("G8", [B, 8], dt.float32)
    BHALF = nc.alloc_sbuf_tensor("BHALF", [B, 3], dt.float32)
    BLO = nc.alloc_sbuf_tensor("BLO", [B, 3], dt.float32)
    BHI = nc.alloc_sbuf_tensor("BHI", [B, 3], dt.float32)
    T1M = nc.alloc_sbuf_tensor("T1M", [B, 3 * TOP], dt.float32)
    T2M = nc.alloc_sbuf_tensor("T2M", [B, 3 * TOP], dt.float32)
    DIF = nc.alloc_sbuf_tensor("DIF", [B, 3 * TOP], dt.float32)
    INT2 = nc.alloc_sbuf_tensor("INT2", [B, TOP], dt.float32)
    INTER = nc.alloc_sbuf_tensor("INTER", [B, TOP], dt.float32)
    AA = nc.alloc_sbuf_tensor("AA", [B, TOP], dt.float32)
    RR = nc.alloc_sbuf_tensor("RR", [B, TOP], dt.float32)
    SUP = nc.alloc_sbuf_tensor("SUP", [B, TOP], dt.float32)
    SUPM = nc.alloc_sbuf_tensor("SUPM", [B, TOP], dt.uint8)
    VV = nc.alloc_sbuf_tensor("VV", [B, 1], dt.float32)
    X = nc.alloc_sbuf_tensor("X", [B, 8], dt.float32)
    D = nc.alloc_sbuf_tensor("D", [B, NMSK * 8], dt.float32)
    OUTT = nc.alloc_sbuf_tensor("OUTT", [B, 60 * 8], dt.float32)

    semD = nc.alloc_semaphore("semD")   # small/critical DMA completions (16 each)
    semB = nc.alloc_semaphore("semB")   # bulk DG DMA completions (16 each)
    semV = nc.alloc_semaphore("semV")   # DVE milestones
    semG = nc.alloc_semaphore("semG")   # gpsimd milestones
    semA = nc.alloc_semaphore("semA")   # ACT milestone

    ctr = {"d": 0, "b": 0}
    marks = {}

    def dma(eng, out_ap, in_ap, sem=semD, key="d"):
        eng.dma_start(out=out_ap, in_=in_ap).then_inc(sem, 16)
        ctr[key] += 16

    def dg_load_boxes(eng, call):
        """Load DG channel rows 0..5 with off/sh for images 8*call..8*call+7."""
        for c in range(3):
            dma(eng, DG[c : 128 : 16, :], off[8 * call : 8 * call + 8, c, :], semB, "b")
            dma(eng, DG[3 + c : 128 : 16, :], sh[8 * call : 8 * call + 8, c, :], semB, "b")

    def wrapped(dram_ap_rows):
        # [8, 64] rows -> indirect_copy's wrapped index layout [8, 16, 4]
        return dram_ap_rows.rearrange("m (r j) -> m r j", r=16)

    with nc.Block() as block:

        @block.gpsimd
        def _(g):
            # inputs + consts
            dma(g, T1[:], cls[:].rearrange("b (q j) -> q b j", q=Q))
            dma(g, CHB[:], chb[:])
            dma(g, JCT[:], jc[:])
            dma(g, PP2T[:], pp2[:])
            marks["d_in"] = ctr["d"]
            # anchor channel rows: loaded once, survive box-row reloads
            for c in range(3):
                dma(g, DG[6 + c : 128 : 16, :], anc[:, c, :], semB, "b")
            dg_load_boxes(g, 0)

            # stage-1 results -> pool layouts (via DRAM bounce)
            g.wait_ge(semV, 1)
            dma(g, scr_vj[:], VJ[:].rearrange("q (b k) -> q b k", b=B))
            dma(g, scr_gi[:], GIDXF[:].rearrange("q (b k) -> q b k", b=B))
            g.wait_ge(semD, ctr["d"])
            dma(g, POOL[:], scr_vj[:].rearrange("q b k -> b q k"))
            dma(g, GIP[:], scr_gi[:].rearrange("q b k -> b q k"))
            g.wait_ge(semD, ctr["d"])
            dma(g, scr_p0[:], POOL[:])      # original pool values for the gather
            dma(g, scr_gip[:], GIP[:])      # pool-parallel global indices
            marks["d_pool"] = ctr["d"]

            # stage-2 results: wrapped top-64 pool positions
            g.wait_ge(semV, 2)
            dma(g, scr_posw[:], POSW[:])
            g.wait_ge(semD, ctr["d"])

            # call #1: gather (value, gidx) pool records at top-64 positions
            for c in range(4):
                dma(g, GD[0:128:16, :], scr_p0[8 * c : 8 * c + 8, :])
                dma(g, GD[1:128:16, :], scr_gip[8 * c : 8 * c + 8, :])
                dma(g, PW1[:], wrapped(scr_posw[8 * c : 8 * c + 8, :]))
                g.wait_ge(semD, ctr["d"])
                ic = g.indirect_copy(OUT1[:, c * TOP : (c + 1) * TOP], GD[:], PW1[:], True)
            dma(g, scr_o1[:], OUT1[:])
            g.wait_ge(semD, ctr["d"])
            o1v = scr_o1[:].rearrange("(g w) (c k) -> c g w k", w=16, c=4)
            dma(g, CV[:], o1v[:, :, 0:1, :])        # values (logits)
            dma(g, GIDX64F[:], o1v[:, :, 1:2, :])   # global indices (f32)
            marks["d_cv"] = ctr["d"]

            # DVE wraps the gidx list; call #2 gathers box channels
            g.wait_ge(semV, 3)
            dma(g, scr_gw[:], GIDXW[:])
            g.wait_ge(semD, ctr["d"])
            for c in range(4):
                dma(g, PW2[:], wrapped(scr_gw[8 * c : 8 * c + 8, :]))
                g.wait_ge(semB, ctr["b"])
                g.wait_ge(semD, ctr["d"])
                g.indirect_copy(G2[:], DG[:], PW2[:], True)
                dma(g, scr_g2[c, :, :], G2[:])
                if c + 1 <= 3:
                    dg_load_boxes(g, c + 1)
            g.wait_ge(semD, ctr["d"])
            dma(g, RAW[:], scr_g2[:].rearrange("c (g w) k -> c g w k", w=16)[:, :, 0:9, :])
            marks["d_raw"] = ctr["d"]

            # output
            g.wait_ge(semV, 4)
            dma(g, outp[:], OUTT[:])
            if dbg:
                dma(g, dbg_outs["d_v1"][:], V1[:])
                dma(g, dbg_outs["d_kp"][:], KP[:])
                dma(g, dbg_outs["d_vj"][:], VJ[:])
                dma(g, dbg_outs["d_gidxf"][:], GIDXF[:])
                dma(g, dbg_outs["d_pool0"][:], scr_p0[:])
                dma(g, dbg_outs["d_gip"][:], scr_gip[:])
                dma(g, dbg_outs["d_vtop"][:], VTOP[:])
                dma(g, dbg_outs["d_posl"][:], POSL[:])
                dma(g, dbg_outs["d_cv"][:], CV[:])
                dma(g, dbg_outs["d_g64"][:], GIDX64F[:])
                dma(g, dbg_outs["d_raw"][:], RAW[:])
                dma(g, dbg_outs["d_gs"][:], GS[:])
            g.wait_ge(semD, ctr["d"])
            g.wait_ge(semB, ctr["b"])

        @block.vector
        def _(v):
            zb_full = Z1[:, 0:1].broadcast_to((128, CH))

            def gap():
                # DVE output writes become visible only after the pipe drains
                # (~266ns); an explicit drain fences short-op RAW hazards.
                v.drain()

            # ---- stage 1: per-chunk top-8 values + exact (value, position) records ----
            v.wait_ge(semD, marks["d_in"])
            v.memset(Z1[:], 0.0)
            for b in range(B):
                v.max(V1[:, b * 8 : (b + 1) * 8], T1[:, b * CH : (b + 1) * CH])
            for b in range(B):
                v.match_replace(T1R[:, b * CH : (b + 1) * CH], V1[:, b * 8 : (b + 1) * 8],
                                T1[:, b * CH : (b + 1) * CH], NEGINF)
            v.tensor_tensor(MKU8[:], T1R[:], T1[:], Alu.not_equal)
            # positions: keys (107-j) at marked cells, -1000 elsewhere; top-8 desc = positions asc
            v.memset(WRK[:], -1000.0)
            v.copy_predicated(WRK[:], MKU8[:], JCT[:])
            for b in range(B):
                v.max(KP[:, b * 8 : (b + 1) * 8], WRK[:, b * CH : (b + 1) * CH])
            gap()
            v.tensor_scalar(GIDXF[:], KP[:], -1.0, 107.0, Alu.mult, Alu.add)   # j
            gap()
            v.tensor_scalar(GIDXF[:], GIDXF[:], CHB[:, 0:1], None, Alu.add)    # + q*108
            # prefix counts of marks per chunk
            for b in range(B):
                v.tensor_tensor_scan(JCT[:, b * CH : (b + 1) * CH], MKU8[:, b * CH : (b + 1) * CH],
                                     zb_full, 0.0, Alu.add, Alu.add)
            # masked values
            v.memset(WRK[:], 0.0)
            v.copy_predicated(WRK[:], MKU8[:], T1[:])
            # value of the c-th marked cell per chunk (exactly one nonzero term)
            vj3 = VJ[:].rearrange("q (b k) -> q b k", k=8)
            t1r3 = T1R[:].rearrange("q (b j) -> q b j", b=B)
            for c in range(8):
                v.scalar_tensor_tensor(T1R[:], JCT[:], float(c + 1), WRK[:], Alu.is_equal, Alu.mult)
                v.tensor_reduce(vj3[:, :, c : c + 1], t1r3, Ax.X, Alu.add)
            gap()
            v.memset(DMY[:, 0:1], 0.0).then_inc(semV, 1)

            # ---- stage 2: per-image top-64 by value, then positions ----
            v.wait_ge(semD, marks["d_pool"])
            for r in range(8):
                v.max(VTOP[:, r * 8 : (r + 1) * 8], POOL[:])
                gap()
                v.match_replace(POOL[:], VTOP[:, r * 8 : (r + 1) * 8], POOL[:], NEGINF)
            # integer position keys at extracted cells
            gap()
            v.tensor_scalar(MD2[:], POOL[:], NEGINF, None, Alu.is_equal)
            gap()
            v.tensor_tensor(K2[:], PP2T[:], MD2[:], Alu.mult)
            gap()
            v.tensor_scalar(K2[:], K2[:], 4096.0, None, Alu.subtract)
            gap()
            for r in range(8):
                v.max(KT[:, r * 8 : (r + 1) * 8], K2[:])
                gap()
                v.match_replace(K2[:], KT[:, r * 8 : (r + 1) * 8], K2[:], NEGINF)
            gap()
            v.tensor_scalar(POSL[:], KT[:], -1.0, 2000.0, Alu.mult, Alu.add)   # pos asc
            gap()
            v.tensor_copy(POSW[:].rearrange("m (r j) -> m r j", j=4),
                          POSL[:].rearrange("m (j r) -> m r j", r=16))
            gap()
            v.memset(DMY[:, 0:1], 0.0).then_inc(semV, 1)

            # ---- candidate list: wrap gidx for call #2; build W (logits) ----
            v.wait_ge(semD, marks["d_cv"])
            v.tensor_copy(GIDXW[:].rearrange("m (r j) -> m r j", j=4),
                          GIDX64F[:].rearrange("m (j r) -> m r j", r=16))
            gap()
            v.memset(DMY[:, 0:1], 0.0).then_inc(semV, 1)

            v.memset(NEGT[:], NEG)
            v.memset(X[:, 0:1], 1.0)
            v.tensor_copy(W[:], CV[:])
            v.tensor_scalar(MU8[:], CV[:], L0, None, Alu.is_le)
            gap()
            v.copy_predicated(W[:], MU8[:], NEGT[:])
            # restrict to exactly the top 60 of 64 (ties by ascending gidx)
            v.tensor_scalar(GT[:], CV[:], VTOP[:, 59:60], None, Alu.is_gt)
            v.tensor_scalar(EQ[:], CV[:], VTOP[:, 59:60], None, Alu.is_equal)
            gap()
            v.tensor_tensor_scan(CUM[:], EQ[:], Z1[0:B, 0:1].broadcast_to((B, TOP)), 0.0, Alu.add, Alu.add)
            v.tensor_reduce(NG[:], GT[:], Ax.X, Alu.add)
            gap()
            v.tensor_scalar(NEED[:], NG[:], -1.0, 60.0, Alu.mult, Alu.add)
            gap()
            v.tensor_scalar(OKE[:], CUM[:], NEED[:, 0:1], None, Alu.is_le)
            gap()
            v.tensor_tensor(KEEP[:], EQ[:], OKE[:], Alu.mult)
            gap()
            v.tensor_tensor(KEEP[:], KEEP[:], GT[:], Alu.add)
            gap()
            v.tensor_scalar(MU8[:], KEEP[:], 0.5, None, Alu.is_lt)
            gap()
            v.copy_predicated(W[:], MU8[:], NEGT[:])

            # ---- decode gathered channels ----
            v.wait_ge(semD, marks["d_raw"])
            v.tensor_tensor(GS[:, 0 : 3 * TOP], RAW[:, 0 : 3 * TOP], RAW[:, 6 * TOP : 9 * TOP], Alu.add)
            v.tensor_scalar(GS[:, 0 : 3 * TOP], GS[:, 0 : 3 * TOP], 4.0, None, Alu.mult)
            v.tensor_copy(GS[:, 3 * TOP : 6 * TOP], RAW[:, 3 * TOP : 6 * TOP])
            v.tensor_tensor(GS[:, 6 * TOP : 7 * TOP], RAW[:, 3 * TOP : 4 * TOP], RAW[:, 4 * TOP : 5 * TOP], Alu.mult)
            v.tensor_tensor(GS[:, 6 * TOP : 7 * TOP], GS[:, 6 * TOP : 7 * TOP], RAW[:, 5 * TOP : 6 * TOP], Alu.mult)
            v.tensor_scalar(HALF[:], GS[:, 3 * TOP : 6 * TOP], 0.5, None, Alu.mult)
            v.tensor_tensor(LOT[:], GS[:, 0 : 3 * TOP], HALF[:], Alu.subtract)
            v.tensor_tensor(HIT[:], GS[:, 0 : 3 * TOP], HALF[:], Alu.add)
            v.wait_ge(semA, 1)   # GS sigmoid channel (ACT)

            hit3 = HIT[:].rearrange("b (c k) -> b c k", c=3)
            lot3 = LOT[:].rearrange("b (c k) -> b c k", c=3)
            v2v = GS[:, 6 * TOP : 7 * TOP]
            zb64 = Z1[0:B, 0:1].broadcast_to((B, TOP))

            # ---- NMS: 20 lockstep steps on logits ----
            for s in range(NMSK):
                v.max(M8[:], W[:])
                gap()
                v.tensor_scalar(OHR[:], W[:], M8[:, 0:1], None, Alu.is_equal)
                gap()
                v.tensor_tensor_scan(CSOH[:], OHR[:], zb64, 0.0, Alu.add, Alu.add)
                gap()
                v.tensor_scalar(CSOH[:], CSOH[:], 1.0, None, Alu.is_equal)
                gap()
                v.tensor_tensor(OH[:], OHR[:], CSOH[:], Alu.mult)
                gap()
                ohb = OH[:].rearrange("b (o k) -> b o k", o=1).broadcast_to((B, 8, TOP))
                v.tensor_tensor(TMP8[:], GS[:], ohb, Alu.mult)
                gap()
                v.tensor_reduce(G8[:], TMP8[:].rearrange("b (c k) -> b c k", c=8), Ax.X, Alu.add)
                gap()
                v.tensor_scalar(BHALF[:], G8[:, 3:6], 0.5, None, Alu.mult)
                gap()
                v.tensor_tensor(BLO[:], G8[:, 0:3], BHALF[:], Alu.subtract)
                v.tensor_tensor(BHI[:], G8[:, 0:3], BHALF[:], Alu.add)
                gap()
                bhib = BHI[:].rearrange("b (c o) -> b c o", o=1).broadcast_to((B, 3, TOP))
                blob = BLO[:].rearrange("b (c o) -> b c o", o=1).broadcast_to((B, 3, TOP))
                v.tensor_tensor(T1M[:].rearrange("b (c k) -> b c k", c=3), hit3, bhib, Alu.min)
                v.tensor_tensor(T2M[:].rearrange("b (c k) -> b c k", c=3), lot3, blob, Alu.max)
                gap()
                v.tensor_tensor(DIF[:], T1M[:], T2M[:], Alu.subtract)
                gap()
                v.tensor_scalar(DIF[:], DIF[:], 0.0, None, Alu.max)
                gap()
                v.tensor_tensor(INT2[:], DIF[:, 0:TOP], DIF[:, TOP : 2 * TOP], Alu.mult)
                gap()
                v.tensor_tensor(INTER[:], INT2[:], DIF[:, 2 * TOP : 3 * TOP], Alu.mult)
                v.tensor_scalar(AA[:], v2v, G8[:, 6:7], -THP, Alu.add, Alu.mult)
                gap()
                v.tensor_tensor(RR[:], INTER[:], AA[:], Alu.add)
                gap()
                v.tensor_scalar(SUP[:], RR[:], 0.0, None, Alu.is_gt)
                gap()
                v.tensor_tensor(SUPM[:], SUP[:], OH[:], Alu.add)
                gap()
                v.copy_predicated(W[:], SUPM[:], NEGT[:])
                v.tensor_scalar(VV[:], M8[:, 0:1], -5e8, None, Alu.is_gt)
                v.tensor_copy(X[:, 1:2], G8[:, 7:8])
                v.tensor_copy(X[:, 2:8], G8[:, 0:6])
                gap()
                v.tensor_scalar(D[:, s * 8 : (s + 1) * 8], X[:], 1.0, VV[:, 0:1], Alu.add, Alu.mult)

            v.tensor_scalar(OUTT[:, 0 : NMSK * 8], D[:], 1.0, None, Alu.subtract)
            v.memset(OUTT[:, NMSK * 8 : 60 * 8], -1.0)
            gap()
            v.memset(DMY[:, 0:1], 0.0).then_inc(semV, 1)

        @block.scalar
        def _(a):
            a.wait_ge(semD, marks["d_cv"])
            a.activation(GS[:, 7 * TOP : 8 * TOP], CV[:], AF.Sigmoid).then_inc(semA, 1)

    return nc


_NC_CACHE = {}


def _get_nc():
    if "nc" not in _NC_CACHE:
        _NC_CACHE["nc"] = build_nc()
    return _NC_CACHE["nc"]


def _host_consts():
    n = np.arange(N)
    a3 = np.stack([n // 576, (n // 24) % 24, n % 24]).astype(np.float32)  # [3, N] zyx
    anc = np.broadcast_to(a3, (8, 3, N)).copy()
    chb = (np.arange(128, dtype=np.float32) * CH).reshape(128, 1)
    jcv = 107.0 - (np.arange(B * CH) % CH).astype(np.float32)
    jc = np.broadcast_to(jcv, (128, B * CH)).copy().astype(np.float32)
    pp2 = np.broadcast_to(6096.0 - np.arange(Q * 8, dtype=np.float32), (B, Q * 8)).copy()
    return anc, chb, jc, pp2


def kernel(cls_out, shape_out, offset_out):
    nc = _get_nc()
    cls = np.ascontiguousarray(cls_out.reshape(256, N), dtype=np.float32)
    off = np.ascontiguousarray(offset_out.reshape(256, 3, N), dtype=np.float32)
    sh = np.ascontiguousarray(shape_out.reshape(256, 3, N), dtype=np.float32)
    anc, chb, jc, pp2 = _host_consts()
    in_maps = []
    for i in range(8):
        s = slice(i * B, (i + 1) * B)
        in_maps.append(
            {"cls": cls[s], "off": off[s], "sh": sh[s], "anc": anc, "chb": chb,
             "jc": jc, "pp2": pp2}
        )
    res = run_bass_kernel_spmd(nc, in_maps, core_ids=list(range(8)))
    out = np.concatenate([res.results[i]["out"] for i in range(8)], axis=0)
    return out.astype(np.float32)
